# revision 10
# baseline (speedup 1.0000x reference)
"""Trainium2 Bass kernel for nn_Dot_Attention (sparse attention softmax).

Computes, for each mention m:
    alpha[m, s] = (queries[m] . values[m, s]) / sqrt(D)
    valid[m, s] = (s < len[m]) & ~(start[m] <= s < end[m])
    out[m, :]   = softmax(where(valid, alpha, -inf))

Sharding: mention dim (axis 0) split across 8 NeuronCores, but mentions are
first SORTED by sentence length and dealt round-robin to the cores, so all
cores share one compiled module AND each (block, partition) slot has a
statically known s-extent: the kernel only DMAs values[m, s, :] for
s < roundup16(len), cutting HBM traffic to ~77% (lengths ~ U[256,512)).

Per core: 256 mentions = 2 blocks of 128 (partition dim). v tiles are
[128 mentions, 16 s, 384 d] (24 KB contiguous per partition). The dot
products are split across engines to stay under the DMA roofline:
  - DVE: affine_mul_reduce fuses (v*scale)*q multiply AND the reduce over D
    into ONE pass for NF=10 of every 16 s-positions (~505 ns each).
  - Pool (gpsimd): tensor_mul for the other 6; ACT: activation(Copy,
    scale, accum_out) reduces those (~724 ns each incl. fixed overheads).
Mask/softmax epilogue per block: masks built on Pool from an iota row vs
per-partition scalars; alpha is clamped to <=30 and shifted by -300*invalid
in one DVE op, so garbage in never-loaded (p, s) regions (which are always
masked) can never reach exp un-clamped; the holes in alpha are also
explicitly zeroed (Pool memsets) so no uninitialized SBUF is ever read.
One ACT Exp emits row sums via accum_out; normalize = reciprocal + mult.

DMA queues: big v loads ride the SP HWDGE ring (ACT ring stalls the
activation stream); small q/scal/out transfers ride the ACT ring.
"""

import math

import numpy as np

M, S, D = 2048, 512, 384
NCORES = 8
ML = M // NCORES          # mentions per core
BLK = 128                 # mentions per block (partition dim)
NBLK = ML // BLK
SG = 16                   # s-positions per values DMA tile (3 MB per DMA)
NF = 10                   # s-positions per group computed via DVE fused op
SCALE = 1.0 / math.sqrt(D)
BIGC = 300.0              # exp(x - BIGC) == 0.0 in fp32 after the clamp
CLAMP = 30.0              # alpha clamp; valid alphas are O(5)

_NC = {}
_SPEC = [None]            # (Gs, pmins) of the last kernel() call


def _build(rep=1, spec=None):
    """Build+compile the per-core Bass module for the given trim spec
    (Gs: s-groups per block; pmins: first loaded partition per group).
    rep>1 unrolls the whole computation rep times (slope timing)."""
    if spec is None:
        spec = _SPEC[0]
    assert spec is not None, "call kernel() first (spec comes from lengths)"
    Gs, pmins = spec
    key = (rep, Gs, pmins)
    if key in _NC:
        return _NC[key]

    import concourse.bacc as bacc
    import concourse.tile as tile
    import concourse.mybir as mybir

    F32 = mybir.dt.float32
    Op = mybir.AluOpType
    Act = mybir.ActivationFunctionType

    nc = bacc.Bacc(
        "TRN2", target_bir_lowering=False, debug=False, num_devices=NCORES
    )
    q_ap = nc.dram_tensor("queries", [ML, D], F32, kind="ExternalInput").ap()
    v_ap = nc.dram_tensor("values", [ML, S, D], F32, kind="ExternalInput").ap()
    s4_ap = nc.dram_tensor("scal3", [ML, 4], F32, kind="ExternalInput").ap()
    io_ap = nc.dram_tensor("iota", [BLK, S], F32, kind="ExternalInput").ap()
    out_ap = nc.dram_tensor("out", [ML, S], F32, kind="ExternalOutput").ap()

    with tile.TileContext(nc) as tc:
        with (
            tc.tile_pool(name="pv", bufs=3) as pv,
            tc.tile_pool(name="pq", bufs=2) as pq,
            tc.tile_pool(name="pa", bufs=2) as pa,
            tc.tile_pool(name="ps", bufs=2) as ps,
            tc.tile_pool(name="pc", bufs=1) as pc,
        ):
            iota_t = pc.tile([BLK, S], F32)
            nc.sync.dma_start(iota_t[:], io_ap)

            # one-time zero of the v buffers: partition-trimmed DMAs leave
            # the low rows untouched, and compute (which must span the full
            # 0..128 partition range — BIR forbids wide accesses starting at
            # a nonzero partition) reads them; after this they only ever
            # hold old v data, so everything stays finite for the clamp.
            for _ in range(3):
                v_t = pv.tile([BLK, SG, D], F32, tag="v")
                nc.gpsimd.memset(v_t[:], 0.0)

            for b in [bb for _ in range(rep) for bb in range(NBLK)]:
                G = Gs[b % NBLK]
                pmin = pmins[b % NBLK]
                m0 = (b % NBLK) * BLK

                q_t = pq.tile([BLK, D], F32, tag="q")
                nc.scalar.dma_start(q_t[:], q_ap[m0 : m0 + BLK, :])
                sc_t = pq.tile([BLK, 4], F32, tag="sc4")
                nc.scalar.dma_start(sc_t[:], s4_ap[m0 : m0 + BLK, :])

                alpha = pa.tile([BLK, S], F32, tag="alpha")
                # zero the tail no accumulate will write (always masked, but
                # must hold finite values for the clamped exp)
                if G * SG < S:
                    nc.gpsimd.memset(alpha[:, G * SG : S], 0.0)

                for g in range(G):
                    pm = pmin[g]
                    v_t = pv.tile([BLK, SG, D], F32, tag="v")
                    nc.sync.dma_start(
                        v_t[pm:BLK, :, :],
                        v_ap[m0 + pm : m0 + BLK, g * SG : (g + 1) * SG, :],
                    )
                    # DVE: fused multiply+reduce for NF s-positions
                    dump = ps.tile([BLK, NF, D], F32, tag="dump")
                    for j in range(NF):
                        s_idx = g * SG + j
                        nc.vector.affine_mul_reduce(
                            dump[:, j, :],
                            alpha[:, s_idx : s_idx + 1],
                            v_t[:, j, :],
                            q_t[:],
                            SCALE,
                            0.0,
                        )
                    # Pool multiply + ACT accum-reduce for the rest
                    prod2 = ps.tile([BLK, SG - NF, D], F32, tag="prod2")
                    dump2 = ps.tile([BLK, SG - NF, D], F32, tag="dump2")
                    for k in range(SG - NF):
                        nc.gpsimd.tensor_mul(
                            prod2[:, k, :], v_t[:, NF + k, :], q_t[:]
                        )
                    for k in range(SG - NF):
                        s_idx = g * SG + NF + k
                        nc.scalar.activation(
                            dump2[:, k, :],
                            prod2[:, k, :],
                            Act.Copy,
                            bias=0.0,
                            scale=SCALE,
                            accum_out=alpha[:, s_idx : s_idx + 1],
                        )

                # invalid = (iota >= len) | ((iota >= start) & (iota < end))
                mA = ps.tile([BLK, S], F32, tag="mA", bufs=1)
                nc.gpsimd.tensor_scalar(mA[:], iota_t[:], sc_t[:, 0:1], None, Op.is_ge)
                mB = ps.tile([BLK, S], F32, tag="mB", bufs=1)
                nc.gpsimd.tensor_scalar(mB[:], iota_t[:], sc_t[:, 1:2], None, Op.is_ge)
                msp = ps.tile([BLK, S], F32, tag="msp", bufs=1)
                nc.vector.scalar_tensor_tensor(
                    msp[:], iota_t[:], sc_t[:, 2:3], mB[:], op0=Op.is_lt, op1=Op.mult
                )
                inval = ps.tile([BLK, S], F32, tag="inval", bufs=1)
                nc.vector.tensor_tensor(inval[:], mA[:], msp[:], Op.max)
                invalS = ps.tile([BLK, S], F32, tag="invalS")
                nc.gpsimd.tensor_scalar(invalS[:], inval[:], -BIGC, None, Op.mult)

                # am = min(alpha, CLAMP) - BIGC*invalid ; exp + row sums
                am = ps.tile([BLK, S], F32, tag="am")
                nc.vector.scalar_tensor_tensor(
                    am[:], alpha[:], sc_t[:, 3:4], invalS[:], op0=Op.min, op1=Op.add
                )
                expv = pa.tile([BLK, S], F32, tag="expv")
                sums = ps.tile([BLK, 1], F32, tag="sums")
                nc.scalar.activation(
                    expv[:], am[:], Act.Exp, bias=0.0, scale=1.0, accum_out=sums[:]
                )
                recip = ps.tile([BLK, 1], F32, tag="recip")
                nc.vector.reciprocal(recip[:], sums[:])
                outt = pa.tile([BLK, S], F32, tag="outt")
                nc.vector.tensor_scalar(outt[:], expv[:], recip[:], None, Op.mult)
                nc.scalar.dma_start(out_ap[m0 : m0 + BLK, :], outt[:])

    nc.compile()
    _NC[key] = nc
    return nc


def _host_prep(idx, lengths):
    """Per-mention [len, start, end, CLAMP] float32 (cols named scal3 for
    compatibility; 4th column carries the alpha clamp constant)."""
    idx = np.asarray(idx)
    lengths = np.asarray(lengths)
    sent = idx[:, 4].astype(np.int64)
    prefix = np.concatenate(
        [np.zeros(1, np.int64), np.cumsum(lengths.astype(np.int64))[:-1]]
    )
    mlen = lengths[sent].astype(np.float32)
    start = (idx[:, 2].astype(np.int64) - prefix[sent]).astype(np.float32)
    end = (idx[:, 3].astype(np.int64) - prefix[sent]).astype(np.float32)
    clamp = np.full_like(mlen, CLAMP)
    return np.stack([mlen, start, end, clamp], axis=1)  # [M, 4] f32


def _plan(scal4):
    """Sort mentions by length, deal round-robin across cores; derive the
    static DMA trim: per (block, partition) s-extent and per (block,
    s-group) first loaded partition."""
    mlen = scal4[:, 0].astype(np.int64)
    order = np.argsort(mlen, kind="stable")
    ls = mlen[order]                        # ascending
    slot_max = ls[7::8]                     # [ML] max len in each 8-rank slot
    ext = np.minimum((slot_max + SG - 1) // SG * SG, S).astype(np.int64)
    Gs, pmins = [], []
    for b in range(NBLK):
        e = ext[b * BLK : (b + 1) * BLK]    # non-decreasing
        G = int(e[-1]) // SG
        pm = tuple(int(np.searchsorted(e, g * SG, side="right")) for g in range(G))
        Gs.append(G)
        pmins.append(pm)
    return order, (tuple(Gs), tuple(pmins))


def kernel(queries, values, idx, lengths):
    from concourse.bass_utils import run_bass_kernel_spmd

    queries = np.asarray(queries, dtype=np.float32)
    values = np.asarray(values, dtype=np.float32)
    scal4 = _host_prep(idx, lengths)
    order, spec = _plan(scal4)
    _SPEC[0] = spec
    iota = np.ascontiguousarray(
        np.broadcast_to(np.arange(S, dtype=np.float32), (BLK, S))
    )

    nc = _build(1, spec)
    in_maps = []
    sels = []
    for c in range(NCORES):
        sel = order[c::8]                   # slot-ordered mentions for core c
        sels.append(sel)
        in_maps.append(
            {
                "queries": np.ascontiguousarray(queries[sel]),
                "values": np.ascontiguousarray(values[sel]),
                "scal3": np.ascontiguousarray(scal4[sel]),
                "iota": iota,
            }
        )
    res = run_bass_kernel_spmd(nc, in_maps, core_ids=list(range(NCORES)))
    out = np.empty((M, S), dtype=np.float32)
    for c in range(NCORES):
        out[sels[c]] = res.results[c]["out"]
    return out


# revision 20
# speedup vs baseline: 1.8967x; 1.8967x over previous
"""Trainium2 Bass kernel for nn_Dot_Attention (sparse attention softmax).

Computes, for each mention m:
    alpha[m, s] = (queries[m] . values[m, s]) / sqrt(D)
    valid[m, s] = (s < len[m]) & ~(start[m] <= s < end[m])
    out[m, :]   = softmax(where(valid, alpha, -inf))

Sharding: mention dim (axis 0) split across 8 NeuronCores. Mentions are
first SORTED by sentence length and dealt round-robin to the cores, so all
cores share one compiled module and each 128-mention block has a small
static max s-extent: the kernel skips s-tiles beyond a block's max length
entirely (lengths ~ U[256,512) -> ~12.5% fewer tiles and ~12.5% less DVE
work). DMAs are always full-width [0:128] on the SP ring only: measured
1635 GB/s streaming vs 226 GB/s for partition-trimmed transfers and
664 GB/s when alternating two rings.

Per core: 256 mentions = 2 blocks of 128 (partition dim). v tiles are
[128 mentions, 16 s, 384 d] (24 KB contiguous per partition). The whole
dot product runs on DVE via affine_mul_reduce, which fuses the
(v*scale)*q multiply AND the reduce over D into ONE instruction per
s-position measured at 150 ns/[128,384] (~328 G elem/s, 2.5x a plain
fp32 tensor_tensor) -- ~134 us/core for all 896 s-positions, which
pipelines under/with the v stream. Pool's tensor ops measure 885 ns for
the same shape (8-DSP engine), so Pool only gets mask comparisons; ACT
(554 ns/accum-instr) only does the Exp and the final normalize.

Mask/softmax epilogue per block: masks from an iota row vs per-partition
scalars; alpha is clamped to <=30 and shifted by -300*invalid in one DVE
op before a single ACT Exp that also emits row sums via accum_out;
normalize = DVE reciprocal + ACT copy with per-partition scale. The
alpha tail beyond the block extent is memset-zeroed so the clamped exp
sees finite values; those columns are always masked.
"""

import math

import numpy as np

M, S, D = 2048, 512, 384
NCORES = 8
ML = M // NCORES          # mentions per core
BLK = 128                 # mentions per block (partition dim)
NBLK = ML // BLK
SG = 16                   # s-positions per values DMA tile (3 MB per DMA)
SCALE = 1.0 / math.sqrt(D)
BIGC = 300.0              # exp(x - BIGC) == 0.0 in fp32 after the clamp
CLAMP = 30.0              # alpha clamp; valid alphas are O(5)

_NC = {}
_SPEC = [None]            # (Gs, pmins) of the last kernel() call


def _build(rep=1, spec=None):
    """Build+compile the per-core Bass module for the given trim spec
    (Gs: s-groups per block; pmins: first loaded partition per group).
    rep>1 unrolls the whole computation rep times (slope timing)."""
    if spec is None:
        spec = _SPEC[0]
    assert spec is not None, "call kernel() first (spec comes from lengths)"
    Gs, pmins = spec
    key = (rep, Gs, pmins)
    if key in _NC:
        return _NC[key]

    import concourse.bacc as bacc
    import concourse.tile as tile
    import concourse.mybir as mybir

    F32 = mybir.dt.float32
    Op = mybir.AluOpType
    Act = mybir.ActivationFunctionType

    nc = bacc.Bacc(
        "TRN2", target_bir_lowering=False, debug=False, num_devices=NCORES
    )
    q_ap = nc.dram_tensor("queries", [ML, D], F32, kind="ExternalInput").ap()
    v_ap = nc.dram_tensor("values", [ML, S, D], F32, kind="ExternalInput").ap()
    s4_ap = nc.dram_tensor("scal3", [ML, 4], F32, kind="ExternalInput").ap()
    io_ap = nc.dram_tensor("iota", [BLK, S], F32, kind="ExternalInput").ap()
    out_ap = nc.dram_tensor("out", [ML, S], F32, kind="ExternalOutput").ap()

    with tile.TileContext(nc) as tc:
        with (
            tc.tile_pool(name="pv", bufs=3) as pv,
            tc.tile_pool(name="pq", bufs=2) as pq,
            tc.tile_pool(name="pa", bufs=2) as pa,
            tc.tile_pool(name="ps", bufs=2) as ps,
            tc.tile_pool(name="pc", bufs=1) as pc,
        ):
            iota_t = pc.tile([BLK, S], F32)
            nc.sync.dma_start(iota_t[:], io_ap)

            for b in [bb for _ in range(rep) for bb in range(NBLK)]:
                G = Gs[b % NBLK]
                m0 = (b % NBLK) * BLK

                q_t = pq.tile([BLK, D], F32, tag="q")
                nc.scalar.dma_start(q_t[:], q_ap[m0 : m0 + BLK, :])
                sc_t = pq.tile([BLK, 4], F32, tag="sc4")
                nc.scalar.dma_start(sc_t[:], s4_ap[m0 : m0 + BLK, :])

                alpha = pa.tile([BLK, S], F32, tag="alpha")
                # zero the tail no accumulate will write (always masked, but
                # must hold finite values for the clamped exp)
                if G * SG < S:
                    nc.gpsimd.memset(alpha[:, G * SG : S], 0.0)

                for g in range(G):
                    # full-width [0:128] DMA only: partition-trimmed DMAs
                    # fall off the HW fast path (226 vs 1635 GB/s measured)
                    v_t = pv.tile([BLK, SG, D], F32, tag="v")
                    nc.sync.dma_start(
                        v_t[:], v_ap[m0 : m0 + BLK, g * SG : (g + 1) * SG, :]
                    )
                    # DVE: fused multiply+reduce for every s-position (the
                    # custom op measures 150 ns/[128,384] — 2.5x a plain mul)
                    dump = ps.tile([BLK, 4, D], F32, tag="dump", bufs=4)
                    for j in range(SG):
                        s_idx = g * SG + j
                        nc.vector.affine_mul_reduce(
                            dump[:, j % 4, :],
                            alpha[:, s_idx : s_idx + 1],
                            v_t[:, j, :],
                            q_t[:],
                            SCALE,
                            0.0,
                        )

                # invalid = (iota >= len) | ((iota >= start) & (iota < end))
                # comparisons on the idle Pool engine; STT/max only on DVE
                mA = ps.tile([BLK, S], F32, tag="mA", bufs=1)
                nc.gpsimd.tensor_scalar(mA[:], iota_t[:], sc_t[:, 0:1], None, Op.is_ge)
                mB = ps.tile([BLK, S], F32, tag="mB", bufs=1)
                nc.gpsimd.tensor_scalar(mB[:], iota_t[:], sc_t[:, 1:2], None, Op.is_ge)
                msp = ps.tile([BLK, S], F32, tag="msp", bufs=1)
                nc.vector.scalar_tensor_tensor(
                    msp[:], iota_t[:], sc_t[:, 2:3], mB[:], op0=Op.is_lt, op1=Op.mult
                )
                inval = ps.tile([BLK, S], F32, tag="inval", bufs=1)
                nc.vector.tensor_tensor(inval[:], mA[:], msp[:], Op.max)
                invalS = ps.tile([BLK, S], F32, tag="invalS")
                nc.gpsimd.tensor_scalar(invalS[:], inval[:], -BIGC, None, Op.mult)

                # am = min(alpha, CLAMP) - BIGC*invalid ; exp + row sums
                am = ps.tile([BLK, S], F32, tag="am")
                nc.vector.scalar_tensor_tensor(
                    am[:], alpha[:], sc_t[:, 3:4], invalS[:], op0=Op.min, op1=Op.add
                )
                expv = pa.tile([BLK, S], F32, tag="expv")
                sums = ps.tile([BLK, 1], F32, tag="sums")
                nc.scalar.activation(
                    expv[:], am[:], Act.Exp, bias=0.0, scale=1.0, accum_out=sums[:]
                )
                recip = ps.tile([BLK, 1], F32, tag="recip")
                nc.vector.reciprocal(recip[:], sums[:])
                # normalize on ACT (scale rides as a per-partition AP)
                outt = pa.tile([BLK, S], F32, tag="outt")
                nc.scalar.activation(
                    outt[:], expv[:], Act.Copy, bias=0.0, scale=recip[:]
                )
                nc.scalar.dma_start(out_ap[m0 : m0 + BLK, :], outt[:])

    nc.compile()
    _NC[key] = nc
    return nc


def _host_prep(idx, lengths):
    """Per-mention [len, start, end, CLAMP] float32 (cols named scal3 for
    compatibility; 4th column carries the alpha clamp constant)."""
    idx = np.asarray(idx)
    lengths = np.asarray(lengths)
    sent = idx[:, 4].astype(np.int64)
    prefix = np.concatenate(
        [np.zeros(1, np.int64), np.cumsum(lengths.astype(np.int64))[:-1]]
    )
    mlen = lengths[sent].astype(np.float32)
    start = (idx[:, 2].astype(np.int64) - prefix[sent]).astype(np.float32)
    end = (idx[:, 3].astype(np.int64) - prefix[sent]).astype(np.float32)
    clamp = np.full_like(mlen, CLAMP)
    return np.stack([mlen, start, end, clamp], axis=1)  # [M, 4] f32


def _plan(scal4):
    """Sort mentions by length, deal round-robin across cores; derive the
    static DMA trim: per (block, partition) s-extent and per (block,
    s-group) first loaded partition."""
    mlen = scal4[:, 0].astype(np.int64)
    order = np.argsort(mlen, kind="stable")
    ls = mlen[order]                        # ascending
    slot_max = ls[7::8]                     # [ML] max len in each 8-rank slot
    ext = np.minimum((slot_max + SG - 1) // SG * SG, S).astype(np.int64)
    Gs = tuple(
        int(ext[b * BLK : (b + 1) * BLK][-1]) // SG for b in range(NBLK)
    )
    return order, (Gs, ())


def kernel(queries, values, idx, lengths):
    from concourse.bass_utils import run_bass_kernel_spmd

    queries = np.asarray(queries, dtype=np.float32)
    values = np.asarray(values, dtype=np.float32)
    scal4 = _host_prep(idx, lengths)
    order, spec = _plan(scal4)
    _SPEC[0] = spec
    iota = np.ascontiguousarray(
        np.broadcast_to(np.arange(S, dtype=np.float32), (BLK, S))
    )

    nc = _build(1, spec)
    in_maps = []
    sels = []
    for c in range(NCORES):
        sel = order[c::8]                   # slot-ordered mentions for core c
        sels.append(sel)
        in_maps.append(
            {
                "queries": np.ascontiguousarray(queries[sel]),
                "values": np.ascontiguousarray(values[sel]),
                "scal3": np.ascontiguousarray(scal4[sel]),
                "iota": iota,
            }
        )
    res = run_bass_kernel_spmd(nc, in_maps, core_ids=list(range(NCORES)))
    out = np.empty((M, S), dtype=np.float32)
    for c in range(NCORES):
        out[sels[c]] = res.results[c]["out"]
    return out
